# revision 28
# baseline (speedup 1.0000x reference)
"""CSPN 3x3 propagation on 8 trn2 NeuronCores (batch-parallel), bf16.

out[y, x] = sum_{i,j} g[3i+j, y+1, x+1] * hn[y+1-i, x+1-j]
  (center tap i=j=1 uses h0; hn/h0 zero-padded outside [0,H)x[0,W))

All wire traffic is bf16 (host casts f32->bf16 inside kernel(); output
is stored bf16 and upcast on host — rel err ~6e-3 vs the 2e-2 gate).
~10.8 MB per core vs 21.6 MB for the f32 version.

Compute, per chunk of <=126 output rows:
- DVE computes the 9 tap products in 2x bf16 perf mode (~0.8us each;
  2x requires 4-byte-aligned element offsets on EVERY operand, and
  GpSimd compute must stay off — it poisons DVE to 0.25x). The j=1
  taps (t=1, t=7) read hn at an odd offset and run 1x; that is cheaper
  than loading a second copy of hn. Bands 0/1 products come first
  (pairs pre-summed via ONE strided 2-slot add), band 2 last with its
  own pair-sum, so little DVE work remains after the last band lands.
- PE shift-sums 6 streams per strip (q0,q1,p2,p5,qb,p8; shift d=2-band
  via exact 0/1 matrices) into PSUM f32.
- Scalar copies PSUM -> SBUF bf16 (512/512/192 strips).

DMA schedule: three rings (sync HWDGE, scalar HWDGE, gpsimd SWDGE —
measured ~110 B/ns each, all spraying across the 16 SDMA engines).
Transfers are issued in GLOBAL just-in-time order: chunk c+1's first
bands are enqueued before chunk c's last band, so each ring's FIFO
delivers every band right when the DVE needs it and the final chunk's
early bands arrive well before the load stream ends. One full-KL
transfer per band (3D APs spray fine even at 128 rows, 2432 B
descriptors). Stores are emitted late so they never stall a ring.

Layouts (per core, B=1):
  guide -> [3200, 1216] bf16: zero row, then 9 planes of [354, 1216]
           (orig cols 1..1217), zero tail
  hn    -> [368, 1218] bf16: row r = hn[r-1] at cols 1..1216, else 0
  h0    -> [368, 1216] bf16: row r = h0[r-1]
  out   -> [352, 1216] bf16
"""

import numpy as np
import ml_dtypes

import concourse.bacc as bacc
import concourse.mybir as mybir
from concourse import tile
from concourse.ap import AP
from concourse.bass_utils import run_bass_kernel_spmd

BF16 = mybir.dt.bfloat16
F32 = mybir.dt.float32
MUL = mybir.AluOpType.mult
ADD = mybir.AluOpType.add

B, H, W = 8, 352, 1216
HP, WPAD = H + 2, W + 2        # 354, 1218
GROWS = 1 + 9 * HP + 13        # 3200
SROWS = 368
N_CORES = 8
CHUNKS = [(0, 126, 128), (126, 126, 128), (252, 100, 112)]  # (y0, R, KL)
STRIPS = [(0, 512), (512, 512), (1024, 192)]


def make_shift_mats():
    """S_d[k, m] = 1 iff k == m + d, d in {0,1,2}; packed [128, 378] bf16."""
    sm = np.zeros((128, 3 * 126), ml_dtypes.bfloat16)
    for d in range(3):
        for m in range(126):
            sm[m + d, d * 126 + m] = 1.0
    return sm


def prep_core_inputs(guide_b: np.ndarray, hn_b: np.ndarray, h0_b: np.ndarray,
                     sm: np.ndarray) -> dict:
    """guide_b [9, 354, 1218] f32, hn_b/h0_b [352, 1216] f32 -> bf16 dram dict."""
    gp = np.zeros((GROWS, W), ml_dtypes.bfloat16)
    gp[1:1 + 9 * HP] = np.asarray(guide_b, np.float32)[:, :, 1:1 + W].reshape(9 * HP, W)
    hnp = np.zeros((SROWS, WPAD), ml_dtypes.bfloat16)
    hnp[1:1 + H, 1:1 + W] = hn_b
    h0p = np.zeros((SROWS, W), ml_dtypes.bfloat16)
    h0p[1:1 + H, :] = h0_b
    return {"guide": gp, "hn": hnp, "h0": h0p, "smat": sm}


def build():
    nc = bacc.Bacc(enable_partition_id=False)
    g_d = nc.dram_tensor("guide", [GROWS, W], BF16, kind="ExternalInput")
    hn_d = nc.dram_tensor("hn", [SROWS, WPAD], BF16, kind="ExternalInput")
    h0_d = nc.dram_tensor("h0", [SROWS, W], BF16, kind="ExternalInput")
    sm_d = nc.dram_tensor("smat", [128, 3 * 126], BF16, kind="ExternalInput")
    out_d = nc.dram_tensor("out", [H, W], BF16, kind="ExternalOutput")

    with tile.TileContext(nc) as tc:
        with tc.tile_pool(name="const", bufs=1) as cpool, \
             tc.tile_pool(name="gpool", bufs=3) as gpool, \
             tc.tile_pool(name="spool", bufs=3) as spool, \
             tc.tile_pool(name="ppool", bufs=2) as ppool, \
             tc.tile_pool(name="opool", bufs=2) as opool, \
             tc.tile_pool(name="psum", bufs=2, space="PSUM") as pspool:

            smt = cpool.tile([128, 3 * 126], BF16)
            nc.gpsimd.dma_start(out=smt[:, :], in_=sm_d[:, :])

            st = {}   # per-chunk tiles

            def issue_smalls(ci):
                y0, R, KL = CHUNKS[ci]
                hnt = spool.tile([128, WPAD], BF16, tag="hn", name="hnt")
                h0t = spool.tile([128, W], BF16, tag="h0", name="h0t")
                if KL == 128:
                    nc.sync.dma_start(out=hnt[0:64, :], in_=hn_d[y0:y0 + 64, :])
                    nc.scalar.dma_start(out=hnt[64:128, :],
                                        in_=hn_d[y0 + 64:y0 + 128, :])
                    nc.gpsimd.dma_start(out=h0t[0:KL, :], in_=h0_d[y0:y0 + KL, :])
                else:
                    nc.sync.dma_start(out=hnt[0:KL, :], in_=hn_d[y0:y0 + KL, :])
                    nc.gpsimd.dma_start(out=h0t[0:KL, :], in_=h0_d[y0:y0 + KL, :])
                st[ci] = {"hnt": hnt, "h0t": h0t}

            rings = [nc.sync, nc.scalar, nc.gpsimd]

            def issue_band(ci, a):
                """Guide band a: planes 3a..3a+2, tile row k <- flat row
                1 + (3a+p)*HP + y0 + a - 1 + k."""
                y0, R, KL = CHUNKS[ci]
                if a == 0:
                    st[ci]["gt"] = gpool.tile([128, 9, W], BF16, tag="g", name="gt")
                gt = st[ci]["gt"]
                base = 1 + 3 * a * HP + y0 + a - 1
                rings[a].dma_start(
                    out=gt[0:KL, 3 * a:3 * a + 3, :],
                    in_=AP(g_d, base * W, [[W, KL], [HP * W, 3], [1, W]]))

            def compute(ci):
                y0, R, KL = CHUNKS[ci]
                hnt, h0t, gt = st[ci]["hnt"], st[ci]["h0t"], st[ci]["gt"]

                def src_for(t):
                    i, j = t // 3, t % 3
                    if t == 4:
                        return h0t[0:KL, :]
                    # j=1 taps read hn at odd offset 1 (1x DVE, still correct)
                    return hnt[0:KL, 2 - j:2 - j + W]

                pt = ppool.tile([128, 9, W], BF16, tag="p", name="pt")
                q = ppool.tile([128, 2, W], BF16, tag="q", name="qt")
                qb = ppool.tile([128, W], BF16, tag="qb", name="qbt")
                for t in (0, 1, 2, 3, 4, 5):
                    nc.vector.tensor_tensor(pt[0:KL, t], gt[0:KL, t], src_for(t), MUL)
                nc.vector.tensor_tensor(
                    q[0:KL],
                    AP(pt.tensor, 0, [[9 * W, KL], [3 * W, 2], [1, W]]),
                    AP(pt.tensor, W, [[9 * W, KL], [3 * W, 2], [1, W]]),
                    ADD)
                for t in (6, 7):
                    nc.vector.tensor_tensor(pt[0:KL, t], gt[0:KL, t], src_for(t), MUL)
                nc.vector.tensor_tensor(qb[0:KL], pt[0:KL, 6], pt[0:KL, 7], ADD)
                nc.vector.tensor_tensor(pt[0:KL, 8], gt[0:KL, 8], src_for(8), MUL)

                psts = [pspool.tile([126, 512], F32, tag=f"ps{s}", name=f"ps{s}")
                        for s in range(len(STRIPS))]
                streams = [(q[0:KL, 0, :], 2), (q[0:KL, 1, :], 1),
                           (pt[0:KL, 2, :], 2), (pt[0:KL, 5, :], 1),
                           (qb[0:KL, :], 0), (pt[0:KL, 8, :], 0)]
                for mi, (mv, d) in enumerate(streams):
                    for s, (w0, N) in enumerate(STRIPS):
                        nc.tensor.matmul(psts[s][0:R, 0:N],
                                         smt[0:KL, d * 126:d * 126 + R],
                                         mv[:, w0:w0 + N],
                                         start=(mi == 0), stop=(mi == 5))

                ot = opool.tile([128, W], BF16, tag="out", name="ot")
                for s, (w0, N) in enumerate(STRIPS):
                    nc.scalar.copy(out=ot[0:R, w0:w0 + N], in_=psts[s][0:R, 0:N])
                st[ci]["ot"] = ot

            def store(ci, ring=None):
                y0, R, KL = CHUNKS[ci]
                ot = st[ci]["ot"]
                if ring is not None:
                    ring.dma_start(out=out_d[y0:y0 + R, :], in_=ot[0:R, :])
                else:
                    for s, (w0, N) in enumerate(STRIPS):
                        rings[(s + 2) % 3].dma_start(
                            out=out_d[y0:y0 + R, w0:w0 + N],
                            in_=ot[0:R, w0:w0 + N])

            # global just-in-time issue order; round-robin rings per band
            issue_smalls(0)
            issue_band(0, 0)
            issue_band(0, 1)
            issue_smalls(1)
            issue_band(0, 2)
            issue_band(1, 0)
            issue_band(1, 1)
            issue_smalls(2)
            compute(0)
            issue_band(1, 2)
            issue_band(2, 0)
            issue_band(2, 1)
            store(0, ring=nc.gpsimd)
            compute(1)
            issue_band(2, 2)
            store(1, ring=nc.gpsimd)
            compute(2)
            store(2)

    nc.finalize()
    return nc


_nc_cache = {}


def _get_nc():
    if "nc" not in _nc_cache:
        _nc_cache["nc"] = build()
    return _nc_cache["nc"]


def kernel(guide_weight: np.ndarray, hn: np.ndarray, h0: np.ndarray) -> np.ndarray:
    """Full inputs: guide_weight [8,9,354,1218], hn/h0 [8,1,352,1216] f32.
    Returns [8,1,352,1216] f32."""
    nc = _get_nc()
    sm = make_shift_mats()
    in_maps = [prep_core_inputs(guide_weight[b], hn[b, 0], h0[b, 0], sm)
               for b in range(B)]
    res = run_bass_kernel_spmd(nc, in_maps, list(range(N_CORES)))
    out = np.stack([np.asarray(res.results[b]["out"]) for b in range(B)], axis=0)
    return out[:, None].astype(np.float32)


# revision 29
# speedup vs baseline: 1.0513x; 1.0513x over previous
"""CSPN 3x3 propagation on 8 trn2 NeuronCores (batch-parallel), bf16.

out[y, x] = sum_{i,j} g[3i+j, y+1, x+1] * hn[y+1-i, x+1-j]
  (center tap i=j=1 uses h0; hn/h0 zero-padded outside [0,H)x[0,W))

All wire traffic is bf16 (host casts f32->bf16 inside kernel(); output
is stored bf16 and upcast on host — rel err ~6e-3 vs the 2e-2 gate).
~10.8 MB per core vs 21.6 MB for the f32 version.

Compute, per chunk of <=126 output rows:
- DVE computes the 9 tap products in 2x bf16 perf mode (~0.8us each;
  2x requires 4-byte-aligned element offsets on EVERY operand, and
  GpSimd compute must stay off — it poisons DVE to 0.25x). The j=1
  taps (t=1, t=7) read hn at an odd offset and run 1x; that is cheaper
  than loading a second copy of hn. Bands 0/1 products come first
  (pairs pre-summed via ONE strided 2-slot add), band 2 last with its
  own pair-sum, so little DVE work remains after the last band lands.
- PE shift-sums 6 streams per strip (q0,q1,p2,p5,qb,p8; shift d=2-band
  via exact 0/1 matrices) into PSUM f32.
- Scalar copies PSUM -> SBUF bf16 (512/512/192 strips).

DMA schedule: three rings (sync HWDGE, scalar HWDGE, gpsimd SWDGE —
measured ~110 B/ns each, all spraying across the 16 SDMA engines).
Transfers are issued in GLOBAL just-in-time order: chunk c+1's first
bands are enqueued before chunk c's last band, so each ring's FIFO
delivers every band right when the DVE needs it and the final chunk's
early bands arrive well before the load stream ends. One full-KL
transfer per band (3D APs spray fine even at 128 rows, 2432 B
descriptors). Stores are emitted late so they never stall a ring.

Layouts (per core, B=1):
  guide -> [3200, 1216] bf16: zero row, then 9 planes of [354, 1216]
           (orig cols 1..1217), zero tail
  hn    -> [368, 1218] bf16: row r = hn[r-1] at cols 1..1216, else 0
  h0    -> [368, 1216] bf16: row r = h0[r-1]
  out   -> [352, 1216] bf16
"""

import numpy as np
import ml_dtypes

import concourse.bacc as bacc
import concourse.mybir as mybir
from concourse import tile
from concourse.ap import AP
from concourse.bass_utils import run_bass_kernel_spmd

BF16 = mybir.dt.bfloat16
F32 = mybir.dt.float32
MUL = mybir.AluOpType.mult
ADD = mybir.AluOpType.add

B, H, W = 8, 352, 1216
HP, WPAD = H + 2, W + 2        # 354, 1218
GROWS = 1 + 9 * HP + 13        # 3200
SROWS = 368
N_CORES = 8
CHUNKS = [(0, 126, 128), (126, 126, 128), (252, 100, 112)]  # (y0, R, KL)
STRIPS = [(0, 512), (512, 512), (1024, 192)]


def make_shift_mats():
    """S_d[k, m] = 1 iff k == m + d, d in {0,1,2}; packed [128, 378] bf16."""
    sm = np.zeros((128, 3 * 126), ml_dtypes.bfloat16)
    for d in range(3):
        for m in range(126):
            sm[m + d, d * 126 + m] = 1.0
    return sm


def prep_core_inputs(guide_b: np.ndarray, hn_b: np.ndarray, h0_b: np.ndarray,
                     sm: np.ndarray) -> dict:
    """guide_b [9, 354, 1218] f32, hn_b/h0_b [352, 1216] f32 -> bf16 dram dict."""
    gp = np.zeros((GROWS, W), ml_dtypes.bfloat16)
    gp[1:1 + 9 * HP] = np.asarray(guide_b, np.float32)[:, :, 1:1 + W].reshape(9 * HP, W)
    hnp = np.zeros((SROWS, WPAD), ml_dtypes.bfloat16)
    hnp[1:1 + H, 1:1 + W] = hn_b
    h0p = np.zeros((SROWS, W), ml_dtypes.bfloat16)
    h0p[1:1 + H, :] = h0_b
    return {"guide": gp, "hn": hnp, "h0": h0p, "smat": sm}


def build():
    nc = bacc.Bacc(enable_partition_id=False)
    g_d = nc.dram_tensor("guide", [GROWS, W], BF16, kind="ExternalInput")
    hn_d = nc.dram_tensor("hn", [SROWS, WPAD], BF16, kind="ExternalInput")
    h0_d = nc.dram_tensor("h0", [SROWS, W], BF16, kind="ExternalInput")
    sm_d = nc.dram_tensor("smat", [128, 3 * 126], BF16, kind="ExternalInput")
    out_d = nc.dram_tensor("out", [H, W], BF16, kind="ExternalOutput")

    with tile.TileContext(nc) as tc:
        with tc.tile_pool(name="const", bufs=1) as cpool, \
             tc.tile_pool(name="gpool", bufs=3) as gpool, \
             tc.tile_pool(name="spool", bufs=3) as spool, \
             tc.tile_pool(name="ppool", bufs=2) as ppool, \
             tc.tile_pool(name="opool", bufs=2) as opool, \
             tc.tile_pool(name="psum", bufs=2, space="PSUM") as pspool:

            smt = cpool.tile([128, 3 * 126], BF16)
            nc.sync.dma_start(out=smt[0:64, :], in_=sm_d[0:64, :])
            nc.scalar.dma_start(out=smt[64:128, :], in_=sm_d[64:128, :])

            st = {}   # per-chunk tiles

            def issue_smalls(ci):
                y0, R, KL = CHUNKS[ci]
                hnt = spool.tile([128, WPAD], BF16, tag="hn", name="hnt")
                h0t = spool.tile([128, W], BF16, tag="h0", name="h0t")
                if KL == 128:
                    nc.sync.dma_start(out=hnt[0:64, :], in_=hn_d[y0:y0 + 64, :])
                    nc.scalar.dma_start(out=hnt[64:128, :],
                                        in_=hn_d[y0 + 64:y0 + 128, :])
                    nc.scalar.dma_start(out=h0t[0:64, :], in_=h0_d[y0:y0 + 64, :])
                    nc.sync.dma_start(out=h0t[64:128, :],
                                      in_=h0_d[y0 + 64:y0 + 128, :])
                else:
                    nc.sync.dma_start(out=hnt[0:KL, :], in_=hn_d[y0:y0 + KL, :])
                    nc.gpsimd.dma_start(out=h0t[0:KL, :], in_=h0_d[y0:y0 + KL, :])
                st[ci] = {"hnt": hnt, "h0t": h0t}

            rings = [nc.sync, nc.scalar, nc.gpsimd]

            def issue_band(ci, a):
                """Guide band a: planes 3a..3a+2, tile row k <- flat row
                1 + (3a+p)*HP + y0 + a - 1 + k."""
                y0, R, KL = CHUNKS[ci]
                if a == 0:
                    st[ci]["gt"] = gpool.tile([128, 9, W], BF16, tag="g", name="gt")
                gt = st[ci]["gt"]
                base = 1 + 3 * a * HP + y0 + a - 1
                rings[a].dma_start(
                    out=gt[0:KL, 3 * a:3 * a + 3, :],
                    in_=AP(g_d, base * W, [[W, KL], [HP * W, 3], [1, W]]))

            def compute(ci):
                y0, R, KL = CHUNKS[ci]
                hnt, h0t, gt = st[ci]["hnt"], st[ci]["h0t"], st[ci]["gt"]

                def src_for(t):
                    i, j = t // 3, t % 3
                    if t == 4:
                        return h0t[0:KL, :]
                    # j=1 taps read hn at odd offset 1 (1x DVE, still correct)
                    return hnt[0:KL, 2 - j:2 - j + W]

                pt = ppool.tile([128, 9, W], BF16, tag="p", name="pt")
                q = ppool.tile([128, 2, W], BF16, tag="q", name="qt")
                qb = ppool.tile([128, W], BF16, tag="qb", name="qbt")
                for t in (0, 1, 2, 3, 4, 5):
                    nc.vector.tensor_tensor(pt[0:KL, t], gt[0:KL, t], src_for(t), MUL)
                nc.vector.tensor_tensor(
                    q[0:KL],
                    AP(pt.tensor, 0, [[9 * W, KL], [3 * W, 2], [1, W]]),
                    AP(pt.tensor, W, [[9 * W, KL], [3 * W, 2], [1, W]]),
                    ADD)
                for t in (6, 7):
                    nc.vector.tensor_tensor(pt[0:KL, t], gt[0:KL, t], src_for(t), MUL)
                nc.vector.tensor_tensor(qb[0:KL], pt[0:KL, 6], pt[0:KL, 7], ADD)
                nc.vector.tensor_tensor(pt[0:KL, 8], gt[0:KL, 8], src_for(8), MUL)

                psts = [pspool.tile([126, 512], F32, tag=f"ps{s}", name=f"ps{s}")
                        for s in range(len(STRIPS))]
                streams = [(q[0:KL, 0, :], 2), (q[0:KL, 1, :], 1),
                           (pt[0:KL, 2, :], 2), (pt[0:KL, 5, :], 1),
                           (qb[0:KL, :], 0), (pt[0:KL, 8, :], 0)]
                for mi, (mv, d) in enumerate(streams):
                    for s, (w0, N) in enumerate(STRIPS):
                        nc.tensor.matmul(psts[s][0:R, 0:N],
                                         smt[0:KL, d * 126:d * 126 + R],
                                         mv[:, w0:w0 + N],
                                         start=(mi == 0), stop=(mi == 5))

                ot = opool.tile([128, W], BF16, tag="out", name="ot")
                for s, (w0, N) in enumerate(STRIPS):
                    nc.scalar.copy(out=ot[0:R, w0:w0 + N], in_=psts[s][0:R, 0:N])
                st[ci]["ot"] = ot

            def store(ci, ring=None):
                y0, R, KL = CHUNKS[ci]
                ot = st[ci]["ot"]
                if ring is not None:
                    ring.dma_start(out=out_d[y0:y0 + R, :], in_=ot[0:R, :])
                else:
                    for s, (w0, N) in enumerate(STRIPS):
                        rings[(s + 2) % 3].dma_start(
                            out=out_d[y0:y0 + R, w0:w0 + N],
                            in_=ot[0:R, w0:w0 + N])

            # global just-in-time issue order; round-robin rings per band
            issue_smalls(0)
            issue_band(0, 0)
            issue_band(0, 1)
            issue_smalls(1)
            issue_band(0, 2)
            issue_band(1, 0)
            issue_band(1, 1)
            issue_smalls(2)
            compute(0)
            issue_band(1, 2)
            issue_band(2, 0)
            issue_band(2, 1)
            store(0, ring=nc.gpsimd)
            compute(1)
            issue_band(2, 2)
            store(1, ring=nc.gpsimd)
            compute(2)
            store(2)

    nc.finalize()
    return nc


_nc_cache = {}


def _get_nc():
    if "nc" not in _nc_cache:
        _nc_cache["nc"] = build()
    return _nc_cache["nc"]


def kernel(guide_weight: np.ndarray, hn: np.ndarray, h0: np.ndarray) -> np.ndarray:
    """Full inputs: guide_weight [8,9,354,1218], hn/h0 [8,1,352,1216] f32.
    Returns [8,1,352,1216] f32."""
    nc = _get_nc()
    sm = make_shift_mats()
    in_maps = [prep_core_inputs(guide_weight[b], hn[b, 0], h0[b, 0], sm)
               for b in range(B)]
    res = run_bass_kernel_spmd(nc, in_maps, list(range(N_CORES)))
    out = np.stack([np.asarray(res.results[b]["out"]) for b in range(B)], axis=0)
    return out[:, None].astype(np.float32)


# revision 30
# speedup vs baseline: 1.0806x; 1.0278x over previous
"""CSPN 3x3 propagation on 8 trn2 NeuronCores (batch-parallel), bf16.

out[y, x] = sum_{i,j} g[3i+j, y+1, x+1] * hn[y+1-i, x+1-j]
  (center tap i=j=1 uses h0; hn/h0 zero-padded outside [0,H)x[0,W))

All wire traffic is bf16 (host casts f32->bf16 inside kernel(); output
is stored bf16 and upcast on host — rel err ~6e-3 vs the 2e-2 gate).
~10.8 MB per core vs 21.6 MB for the f32 version.

Compute, per chunk of <=126 output rows:
- DVE computes the 9 tap products in 2x bf16 perf mode (~0.8us each;
  2x requires 4-byte-aligned element offsets on EVERY operand, and
  GpSimd compute must stay off — it poisons DVE to 0.25x). The j=1
  taps (t=1, t=7) read hn at an odd offset and run 1x; that is cheaper
  than loading a second copy of hn. Bands 0/1 products come first
  (pairs pre-summed via ONE strided 2-slot add), band 2 last with its
  own pair-sum, so little DVE work remains after the last band lands.
- PE shift-sums 6 streams per strip (q0,q1,p2,p5,qb,p8; shift d=2-band
  via exact 0/1 matrices) into PSUM f32.
- Scalar copies PSUM -> SBUF bf16 (512/512/192 strips).

DMA schedule: three rings (sync HWDGE, scalar HWDGE, gpsimd SWDGE —
measured ~110 B/ns each, all spraying across the 16 SDMA engines).
Transfers are issued in GLOBAL just-in-time order: chunk c+1's first
bands are enqueued before chunk c's last band, so each ring's FIFO
delivers every band right when the DVE needs it and the final chunk's
early bands arrive well before the load stream ends. One full-KL
transfer per band (3D APs spray fine even at 128 rows, 2432 B
descriptors). Stores are emitted late so they never stall a ring.

Layouts (per core, B=1):
  guide -> [3200, 1216] bf16: zero row, then 9 planes of [354, 1216]
           (orig cols 1..1217), zero tail
  hn    -> [368, 1218] bf16: row r = hn[r-1] at cols 1..1216, else 0
  h0    -> [368, 1216] bf16: row r = h0[r-1]
  out   -> [352, 1216] bf16
"""

import numpy as np
import ml_dtypes

import concourse.bacc as bacc
import concourse.mybir as mybir
from concourse import tile
from concourse.ap import AP
from concourse.bass_utils import run_bass_kernel_spmd

BF16 = mybir.dt.bfloat16
F32 = mybir.dt.float32
MUL = mybir.AluOpType.mult
ADD = mybir.AluOpType.add

B, H, W = 8, 352, 1216
HP, WPAD = H + 2, W + 2        # 354, 1218
GROWS = 1 + 9 * HP + 13        # 3200
SROWS = 368
N_CORES = 8
CHUNKS = [(0, 126, 128), (126, 126, 128), (252, 100, 112)]  # (y0, R, KL)
STRIPS = [(0, 512), (512, 512), (1024, 192)]


def make_shift_mats():
    """S_d[k, m] = 1 iff k == m + d, d in {0,1,2}; packed [128, 378] bf16."""
    sm = np.zeros((128, 3 * 126), ml_dtypes.bfloat16)
    for d in range(3):
        for m in range(126):
            sm[m + d, d * 126 + m] = 1.0
    return sm


def prep_core_inputs(guide_b: np.ndarray, hn_b: np.ndarray, h0_b: np.ndarray,
                     sm: np.ndarray) -> dict:
    """guide_b [9, 354, 1218] f32, hn_b/h0_b [352, 1216] f32 -> bf16 dram dict."""
    gp = np.zeros((GROWS, W), ml_dtypes.bfloat16)
    gp[1:1 + 9 * HP] = np.asarray(guide_b, np.float32)[:, :, 1:1 + W].reshape(9 * HP, W)
    hnp = np.zeros((SROWS, WPAD), ml_dtypes.bfloat16)
    hnp[1:1 + H, 1:1 + W] = hn_b
    h0p = np.zeros((SROWS, W), ml_dtypes.bfloat16)
    h0p[1:1 + H, :] = h0_b
    return {"guide": gp, "hn": hnp, "h0": h0p, "smat": sm}


def build():
    nc = bacc.Bacc(enable_partition_id=False)
    g_d = nc.dram_tensor("guide", [GROWS, W], BF16, kind="ExternalInput")
    hn_d = nc.dram_tensor("hn", [SROWS, WPAD], BF16, kind="ExternalInput")
    h0_d = nc.dram_tensor("h0", [SROWS, W], BF16, kind="ExternalInput")
    sm_d = nc.dram_tensor("smat", [128, 3 * 126], BF16, kind="ExternalInput")
    out_d = nc.dram_tensor("out", [H, W], BF16, kind="ExternalOutput")

    with tile.TileContext(nc) as tc:
        with tc.tile_pool(name="const", bufs=1) as cpool, \
             tc.tile_pool(name="gpool", bufs=3) as gpool, \
             tc.tile_pool(name="spool", bufs=3) as spool, \
             tc.tile_pool(name="ppool", bufs=3) as ppool, \
             tc.tile_pool(name="opool", bufs=2) as opool, \
             tc.tile_pool(name="psum", bufs=2, space="PSUM") as pspool:

            smt = cpool.tile([128, 3 * 126], BF16)
            nc.sync.dma_start(out=smt[0:64, :], in_=sm_d[0:64, :])
            nc.scalar.dma_start(out=smt[64:128, :], in_=sm_d[64:128, :])

            st = {}   # per-chunk tiles

            def issue_smalls(ci):
                y0, R, KL = CHUNKS[ci]
                hnt = spool.tile([128, WPAD], BF16, tag="hn", name="hnt")
                h0t = spool.tile([128, W], BF16, tag="h0", name="h0t")
                if KL == 128:
                    nc.sync.dma_start(out=hnt[0:64, :], in_=hn_d[y0:y0 + 64, :])
                    nc.scalar.dma_start(out=hnt[64:128, :],
                                        in_=hn_d[y0 + 64:y0 + 128, :])
                    nc.scalar.dma_start(out=h0t[0:64, :], in_=h0_d[y0:y0 + 64, :])
                    nc.sync.dma_start(out=h0t[64:128, :],
                                      in_=h0_d[y0 + 64:y0 + 128, :])
                else:
                    nc.sync.dma_start(out=hnt[0:KL, :], in_=hn_d[y0:y0 + KL, :])
                    nc.gpsimd.dma_start(out=h0t[0:KL, :], in_=h0_d[y0:y0 + KL, :])
                st[ci] = {"hnt": hnt, "h0t": h0t}

            rings = [nc.sync, nc.scalar, nc.gpsimd]

            def issue_band(ci, a):
                """Guide band a: planes 3a..3a+2, tile row k <- flat row
                1 + (3a+p)*HP + y0 + a - 1 + k."""
                y0, R, KL = CHUNKS[ci]
                if a == 0:
                    st[ci]["gt"] = gpool.tile([128, 9, W], BF16, tag="g", name="gt")
                gt = st[ci]["gt"]
                base = 1 + 3 * a * HP + y0 + a - 1
                rings[a].dma_start(
                    out=gt[0:KL, 3 * a:3 * a + 3, :],
                    in_=AP(g_d, base * W, [[W, KL], [HP * W, 3], [1, W]]))

            def compute(ci):
                y0, R, KL = CHUNKS[ci]
                hnt, h0t, gt = st[ci]["hnt"], st[ci]["h0t"], st[ci]["gt"]

                def src_for(t):
                    i, j = t // 3, t % 3
                    if t == 4:
                        return h0t[0:KL, :]
                    # j=1 taps read hn at odd offset 1 (1x DVE, still correct)
                    return hnt[0:KL, 2 - j:2 - j + W]

                pt = ppool.tile([128, 9, W], BF16, tag="p", name="pt")
                q = ppool.tile([128, 2, W], BF16, tag="q", name="qt")
                qb = ppool.tile([128, W], BF16, tag="qb", name="qbt")
                for t in (0, 1, 2, 3, 4, 5):
                    nc.vector.tensor_tensor(pt[0:KL, t], gt[0:KL, t], src_for(t), MUL)
                nc.vector.tensor_tensor(
                    q[0:KL],
                    AP(pt.tensor, 0, [[9 * W, KL], [3 * W, 2], [1, W]]),
                    AP(pt.tensor, W, [[9 * W, KL], [3 * W, 2], [1, W]]),
                    ADD)
                for t in (6, 7):
                    nc.vector.tensor_tensor(pt[0:KL, t], gt[0:KL, t], src_for(t), MUL)
                nc.vector.tensor_tensor(qb[0:KL], pt[0:KL, 6], pt[0:KL, 7], ADD)
                nc.vector.tensor_tensor(pt[0:KL, 8], gt[0:KL, 8], src_for(8), MUL)

                psts = [pspool.tile([126, 512], F32, tag=f"ps{s}", name=f"ps{s}")
                        for s in range(len(STRIPS))]
                streams = [(q[0:KL, 0, :], 2), (q[0:KL, 1, :], 1),
                           (pt[0:KL, 2, :], 2), (pt[0:KL, 5, :], 1),
                           (qb[0:KL, :], 0), (pt[0:KL, 8, :], 0)]
                for mi, (mv, d) in enumerate(streams):
                    for s, (w0, N) in enumerate(STRIPS):
                        nc.tensor.matmul(psts[s][0:R, 0:N],
                                         smt[0:KL, d * 126:d * 126 + R],
                                         mv[:, w0:w0 + N],
                                         start=(mi == 0), stop=(mi == 5))

                ot = opool.tile([128, W], BF16, tag="out", name="ot")
                for s, (w0, N) in enumerate(STRIPS):
                    nc.scalar.copy(out=ot[0:R, w0:w0 + N], in_=psts[s][0:R, 0:N])
                st[ci]["ot"] = ot

            def store(ci, ring=None):
                y0, R, KL = CHUNKS[ci]
                ot = st[ci]["ot"]
                if ring is not None:
                    ring.dma_start(out=out_d[y0:y0 + R, :], in_=ot[0:R, :])
                else:
                    for s, (w0, N) in enumerate(STRIPS):
                        rings[(s + 2) % 3].dma_start(
                            out=out_d[y0:y0 + R, w0:w0 + N],
                            in_=ot[0:R, w0:w0 + N])

            # global just-in-time issue order; round-robin rings per band
            issue_smalls(0)
            issue_band(0, 0)
            issue_band(0, 1)
            issue_smalls(1)
            issue_band(0, 2)
            issue_band(1, 0)
            issue_band(1, 1)
            issue_smalls(2)
            compute(0)
            issue_band(1, 2)
            issue_band(2, 0)
            issue_band(2, 1)
            store(0, ring=nc.gpsimd)
            compute(1)
            issue_band(2, 2)
            store(1, ring=nc.gpsimd)
            compute(2)
            store(2)

    nc.finalize()
    return nc


_nc_cache = {}


def _get_nc():
    if "nc" not in _nc_cache:
        _nc_cache["nc"] = build()
    return _nc_cache["nc"]


def kernel(guide_weight: np.ndarray, hn: np.ndarray, h0: np.ndarray) -> np.ndarray:
    """Full inputs: guide_weight [8,9,354,1218], hn/h0 [8,1,352,1216] f32.
    Returns [8,1,352,1216] f32."""
    nc = _get_nc()
    sm = make_shift_mats()
    in_maps = [prep_core_inputs(guide_weight[b], hn[b, 0], h0[b, 0], sm)
               for b in range(B)]
    res = run_bass_kernel_spmd(nc, in_maps, list(range(N_CORES)))
    out = np.stack([np.asarray(res.results[b]["out"]) for b in range(B)], axis=0)
    return out[:, None].astype(np.float32)
